# revision 11
# baseline (speedup 1.0000x reference)
"""Trainium2 Bass kernel for nn_DLTSolver (planar bf16 rewrite).

The reference solves, per batch element b (B = 1048576), an 8x8 linear
system that collapses analytically to a 2x2 Cramer solve plus affine
back-substitution (pure elementwise math in the 8 shift components
s0..s7):

    q  = s3 - s1            b = s2 - s6         c  = s7 - s5
    a  = (s7+512) - s3      d = (s4-512) - s6
    r1 = (s2+512)*q - (s7+512)
    r2 = s0*s4 + (s6+512) - (s5+512)^2
    det = a*d - b*c ;  inv = 1/(512*det)
    x6 = (r1*d - b*r2)*inv ;  x7 = (a*r2 - c*r1)*inv
    y0 = (s2-s5)/512 - s4 + x6   y1 = (s1-s0)/512 - s3 + x7
    y2 = -1 - s2/512 - x6        y3 = -s1/512 - x7
    y4 =  1 + s5/512 - x6        y5 =  s0/512 - x7
    out = [y0 y1 y2 y3 y4 y5 x6 x7 1] reshaped (3,3)

Layout strategy (the big change vs the interleaved kernel): the host
re-packs the input into PLANAR bf16 component planes, (tile, 128, 8
planes, T) per core, and the device writes planar bf16 output planes
[y0..y5, x6, x7].  The host re-interleaves + upcasts + appends the
constant ones column.  Consequences on device:
  - every engine op is a dense unit-stride plane op (no 32B-strided
    access tax, no (t,9) interleave tax),
  - all-bf16 tensor_tensor ops run in the DVE 2x_1P perf mode,
  - HBM traffic halves: 2 MiB in + 2 MiB out per core,
  - the ones column never touches the device.
Numerically verified in simulation: all-bf16 closed form gives
l2 rel err ~3.2e-3 vs the fp32 reference (gate is 2e-2).

Per-core work: 131072 elements = NT tiles x (128 partitions x T cols).
Engine split per tile: 16 DVE insts (packed 2-plane TT/STT), 6 GPSIMD
TT, 7 ACT (Square / Reciprocal spline / affine Copy), 2 HWDGE DMAs.

Input plane order (host-permuted so the packed 2-wide reads all have
non-negative plane steps): pos -> comp = [s0 s1 s2 s4 s3 s6 s5 s7].
"""

import numpy as np

P = 128
T = 512            # cols per partition per tile
NT = 2             # tiles per core
BC = P * T * NT    # elements per core = 131072
NCORES = 8
B_FULL = BC * NCORES

# plane position -> input component index (chosen so packed reads step >= 0)
XORDER = [0, 1, 2, 4, 3, 6, 5, 7]
# X plane positions by component
XP = {c: p for p, c in enumerate(XORDER)}

_CACHE: dict = {}


def _build_bass(legalize=True):
    import concourse.bass as bass
    import concourse.mybir as mybir
    from concourse.tile import TileContext

    bf16 = mybir.dt.bfloat16
    OP = mybir.AluOpType
    AF = mybir.ActivationFunctionType

    nc = bass.Bass("TRN2", use_seq_codegen=True)
    x = nc.dram_tensor("x", [NT, P, 8, T], bf16, kind="ExternalInput")
    y = nc.dram_tensor("y", [NT, P, 8, T], bf16, kind="ExternalOutput")

    # mid-plane layout (chosen so every packed read/write AP has a
    # non-negative plane step); 2-input tensor ops on DVE only -- GPSIMD
    # tensor_tensor contends with DVE for the shared SBUF port (measured
    # ~2.4x mutual slowdown when both run).  GPSIMD gets only 1-input
    # tensor_scalar ops; ACT gets single-src affine/square/recip.
    NM = 37
    (D_, A_, R2, R1, B_, U_, C_, R1D, BR2, AR2, CR1, AD, BCp, N6, N7,
     DET, INV, INV2, E0, E1, Q_, R1A, T2, W2N, W2S, S6P, S3M, S2P, S7P,
     ES0, ES1, V0, V1, G2, G3, G4, G5) = range(NM)

    def act_recip(eng, out_ap, in_ap, scale):
        f32 = mybir.dt.float32
        eng.add_instruction(mybir.InstActivation(
            name=nc.get_next_instruction_name(),
            func=AF.Reciprocal,
            ins=[eng.lower_ap(in_ap),
                 mybir.ImmediateValue(dtype=f32, value=0.0),
                 mybir.ImmediateValue(dtype=f32, value=scale),
                 mybir.ImmediateValue(dtype=f32, value=0.0)],
            outs=[eng.lower_ap(out_ap)],
        ))

    with TileContext(nc, pool_alloc_mode="queue") as tc:
        with tc.tile_pool(name="io", bufs=3) as io, \
             tc.tile_pool(name="mid", bufs=2) as mid:
            for i in range(NT):
                X = io.tile([P, 8, T], bf16, tag="X", name="X")
                # split the input DMA: planes 0-4 (s0,s1,s2,s4,s3) arrive
                # first and unblock the early ops; 5-7 (s6,s5,s7) follow
                nc.sync.dma_start(out=X[:, 0:5, :], in_=x[i, :, 0:5, :])
                nc.sync.dma_start(out=X[:, 5:8, :], in_=x[i, :, 5:8, :])
                Y = io.tile([P, 8, T], bf16, tag="Y", name="Y")
                M = mid.tile([P, NM, T], bf16, tag="M", name="M")

                def xs(c, w=1, step=1):
                    p = XP[c]
                    return X[:, p:p + 1 + (w - 1) * step:step, :]

                def m(p, w=1, step=1):
                    return M[:, p:p + 1 + (w - 1) * step:step, :]

                def ys(p, w=1):
                    return Y[:, p:p + w, :]

                V = nc.vector
                S = nc.scalar
                G = nc.gpsimd

                # ---- GPSIMD: 1-input tensor_scalar only ----
                G.tensor_scalar(m(G2), xs(2), -1.0 / 512, -1.0,
                                OP.mult, OP.add)
                G.tensor_scalar(m(G3), xs(1), -1.0 / 512, 0.0,
                                OP.mult, OP.add)
                G.tensor_scalar(m(G4), xs(5), 1.0 / 512, 1.0,
                                OP.mult, OP.add)
                G.tensor_scalar(m(G5), xs(0), 1.0 / 512, 0.0,
                                OP.mult, OP.add)

                # ---- ACT: single-src affine / square / recip ----
                # w2s = ((s5+512)/512)^2  (bias=1.0 is a registered const
                # AP); w2n = 512 - 512^2*w2s = 512 - (s5+512)^2
                S.activation(m(W2S), xs(5), AF.Square,
                             bias=1.0, scale=1.0 / 512)
                S.activation(m(W2N), m(W2S), AF.Copy,
                             bias=512.0, scale=-512.0 * 512.0)
                # shift planes so the main chain stays in 2x-mode TT:
                # [s2p, s7p] = [s2, s7] + 512 ; s3m = s3 - 512 ; s6p = s6 + 512
                S.activation(M[:, S2P:S2P + 2, :], xs(2, 2, 5), AF.Copy,
                             bias=512.0, scale=1.0)
                S.activation(m(S3M), xs(3), AF.Copy, bias=-512.0, scale=1.0)
                S.activation(m(S6P), xs(6), AF.Copy, bias=512.0, scale=1.0)

                # ---- DVE: everything else (all-bf16 unit-stride 2x TT) ----
                # [b, c] = [s2, s7] - [s6, s5]
                V.tensor_tensor(m(B_, 2, 2), xs(2, 2, 5), xs(6, 2, 1),
                                OP.subtract)
                # [E1, q] = [s1, s3] - [s0, s1]
                V.tensor_tensor(m(E1, 2, 1), xs(1, 2, 3), xs(0, 2, 1),
                                OP.subtract)
                V.tensor_tensor(m(E0), xs(2), xs(5), OP.subtract)
                V.tensor_tensor(m(T2), xs(0), xs(4), OP.mult)
                # [d, a] = [s4, s7] - [s6p, s3m]
                V.tensor_tensor(m(D_, 2, 1), xs(4, 2, 4), m(S6P, 2, 1),
                                OP.subtract)
                V.tensor_tensor(m(R1A), m(S2P), m(Q_), OP.mult)
                V.tensor_tensor(m(R1), m(R1A), m(S7P), OP.subtract)
                V.tensor_tensor(m(U_), m(T2), xs(6), OP.add)
                # r2 = u + (512 - (s5+512)^2) = s0*s4 + s6 + 512 - (s5+512)^2
                V.tensor_tensor(m(R2), m(U_), m(W2N), OP.add)
                # [ad, bc] = [a, b] * [d, c]
                V.tensor_tensor(m(AD, 2, 1), m(A_, 2, 3), m(D_, 2, 6),
                                OP.mult)
                # [r1d, br2] = [r1, b] * [d, r2]
                V.tensor_tensor(m(R1D, 2, 1), m(R1, 2, 1), m(D_, 2, 2),
                                OP.mult)
                # [ar2, cr1] = [a, c] * [r2, r1]
                V.tensor_tensor(m(AR2, 2, 1), m(A_, 2, 5), m(R2, 2, 1),
                                OP.mult)
                # [n6, n7, det] = [r1d, ar2, ad] - [br2, cr1, bc]
                V.tensor_tensor(m(N6, 3, 1), m(R1D, 3, 2), m(BR2, 3, 2),
                                OP.subtract)
                act_recip(S, m(INV), m(DET), 512.0)
                act_recip(S, m(INV2), m(DET), 512.0)
                # [x6, x7] = [n6, n7] * [inv, inv2]
                V.tensor_tensor(ys(6, 2), m(N6, 2, 1), m(INV, 2, 1),
                                OP.mult)
                # [es0, es1] = [E0, E1]/512  (ACT, frees the STT)
                S.activation(M[:, ES0:ES0 + 2, :], m(E0, 2, 1), AF.Copy,
                             bias=0.0, scale=1.0 / 512)
                # [V0, V1] = [es0, es1] - [s4, s3]
                V.tensor_tensor(m(V0, 2, 1), m(ES0, 2, 1), xs(4, 2, 1),
                                OP.subtract)
                # [y0, y1] = [V0, V1] + [x6, x7]
                V.tensor_tensor(ys(0, 2), m(V0, 2, 1), ys(6, 2), OP.add)
                # [y2..y5] = [g2, g3, g4, g5] - [x6, x7, x6, x7]
                V.tensor_tensor(
                    Y[:, 2:6, :].rearrange("p (a b) t -> p a b t", b=2),
                    M[:, G2:G2 + 4, :].rearrange("p (a b) t -> p a b t", b=2),
                    Y[:, 6:8, :].unsqueeze(1).broadcast_to((P, 2, 2, T)),
                    OP.subtract)

                # split the output DMA so planes stream out as they finish
                nc.sync.dma_start(out=y[i, :, 0:2, :], in_=Y[:, 0:2, :])
                nc.sync.dma_start(out=y[i, :, 2:8, :], in_=Y[:, 2:8, :])
    if legalize:
        _legalize_waits(nc)
    return nc


def _legalize_waits(nc, max_waits=1):
    """HW instructions encode at most one semaphore wait; hoist extras
    onto NoOp carriers in the same engine queue."""
    import concourse.mybir as mybir

    skip = ("InstNoOp",)
    for f in nc.m.functions:
        for blk in f.blocks:
            il = blk.instructions
            out = []
            changed = False
            for inst in il:
                si = inst.sync_info
                if (si is not None and len(si.on_wait) > max_waits
                        and type(inst).__name__ not in skip):
                    waits = list(si.on_wait)
                    for w in waits[:-max_waits]:
                        out.append(mybir.InstNoOp(
                            name=nc.get_next_instruction_name(),
                            engine=inst.engine,
                            bass_nofuse=True,
                            sync_info=mybir.SyncInfo(
                                on_wait=[w], on_update=[]),
                        ))
                    inst.sync_info = mybir.SyncInfo(
                        on_wait=waits[-max_waits:],
                        on_update=list(si.on_update))
                    changed = True
                out.append(inst)
            if changed:
                blk.instructions = out


def _get_nc():
    if "nc" not in _CACHE:
        _CACHE["nc"] = _build_bass()
    return _CACHE["nc"]


def _run(shards, trace=False, **kwargs):
    from concourse.bass_utils import run_bass_kernel_spmd
    nc = _get_nc()
    in_maps = [{"x": s} for s in shards]
    return run_bass_kernel_spmd(
        nc, in_maps, core_ids=list(range(NCORES)), trace=trace, **kwargs)


def _prep_shards(pre_4pt_shift: np.ndarray):
    """(B, 8, 1) fp32 -> per-core planar bf16 shards (NT, P, 8, T)."""
    import ml_dtypes
    xf = np.asarray(pre_4pt_shift, dtype=np.float32).reshape(B_FULL, 8)
    xb = xf[:, XORDER].astype(ml_dtypes.bfloat16)
    xb = xb.reshape(NCORES, NT, P, T, 8).transpose(0, 1, 2, 4, 3)
    return [np.ascontiguousarray(xb[i]) for i in range(NCORES)]


def _assemble(results) -> np.ndarray:
    """per-core planar bf16 y planes -> (B, 3, 3) fp32 with ones col."""
    out = np.empty((B_FULL, 9), dtype=np.float32)
    out[:, 8] = 1.0
    for i in range(NCORES):
        yi = np.asarray(results[i]["y"]).astype(np.float32)  # (NT,P,8,T)
        out[i * BC:(i + 1) * BC, :8] = (
            yi.transpose(0, 1, 3, 2).reshape(BC, 8))
    return out.reshape(B_FULL, 3, 3)


def kernel(pre_4pt_shift: np.ndarray) -> np.ndarray:
    shards = _prep_shards(pre_4pt_shift)
    r = _run(shards)
    return _assemble(r.results)


# revision 18
# speedup vs baseline: 1.1002x; 1.1002x over previous
"""Trainium2 Bass kernel for nn_DLTSolver (planar bf16 rewrite).

The reference solves, per batch element b (B = 1048576), an 8x8 linear
system that collapses analytically to a 2x2 Cramer solve plus affine
back-substitution (pure elementwise math in the 8 shift components
s0..s7):

    q  = s3 - s1            b = s2 - s6         c  = s7 - s5
    a  = (s7+512) - s3      d = (s4-512) - s6
    r1 = (s2+512)*q - (s7+512)
    r2 = s0*s4 + (s6+512) - (s5+512)^2
    det = a*d - b*c ;  inv = 1/(512*det)
    x6 = (r1*d - b*r2)*inv ;  x7 = (a*r2 - c*r1)*inv
    y0 = (s2-s5)/512 - s4 + x6   y1 = (s1-s0)/512 - s3 + x7
    y2 = -1 - s2/512 - x6        y3 = -s1/512 - x7
    y4 =  1 + s5/512 - x6        y5 =  s0/512 - x7
    out = [y0 y1 y2 y3 y4 y5 x6 x7 1] reshaped (3,3)

Layout strategy (the big change vs the interleaved kernel): the host
re-packs the input into PLANAR bf16 component planes, (tile, 128, 8
planes, T) per core, and the device writes planar bf16 output planes
[y0..y5, x6, x7].  The host re-interleaves + upcasts + appends the
constant ones column.  Consequences on device:
  - every engine op is a dense unit-stride plane op (no 32B-strided
    access tax, no (t,9) interleave tax),
  - all-bf16 tensor_tensor ops run in the DVE 2x_1P perf mode,
  - HBM traffic halves: 2 MiB in + 2 MiB out per core,
  - the ones column never touches the device.
Numerically verified in simulation: all-bf16 closed form gives
l2 rel err ~3.2e-3 vs the fp32 reference (gate is 2e-2).

Per-core work: 131072 elements = NT tiles x (128 partitions x T cols).
Engine split per tile: 16 DVE insts (packed 2-plane TT/STT), 6 GPSIMD
TT, 7 ACT (Square / Reciprocal spline / affine Copy), 2 HWDGE DMAs.

Input plane order (host-permuted so the packed 2-wide reads all have
non-negative plane steps): pos -> comp = [s0 s1 s2 s4 s3 s6 s5 s7].
"""

import numpy as np

P = 128
T = 512            # cols per partition per tile
NT = 2             # tiles per core
BC = P * T * NT    # elements per core = 131072
NCORES = 8
B_FULL = BC * NCORES

# plane position -> input component index (chosen so packed reads step >= 0
# and the critical-chain operands s1,s3,s2 ride in the first DMA chunk)
XORDER = [1, 3, 2, 4, 0, 6, 5, 7]
# device output plane order (x7,x6 finish first, then y1,y0, then y3,y2,y5,y4)
YORDER = [7, 6, 1, 0, 3, 2, 5, 4]  # plane position -> h-index
# YINV[h] = plane position holding h
YINV = [YORDER.index(h) for h in range(8)]
# X plane positions by component
XP = {c: p for p, c in enumerate(XORDER)}

_CACHE: dict = {}


def _build_bass(legalize=True):
    import concourse.bass as bass
    import concourse.mybir as mybir
    from concourse.tile import TileContext

    bf16 = mybir.dt.bfloat16
    OP = mybir.AluOpType
    AF = mybir.ActivationFunctionType

    nc = bass.Bass("TRN2", use_seq_codegen=True)
    x = nc.dram_tensor("x", [NT, P, 8, T], bf16, kind="ExternalInput")
    y = nc.dram_tensor("y", [NT, P, 8, T], bf16, kind="ExternalOutput")

    # mid-plane layout (chosen so every packed read/write AP has a
    # non-negative plane step); 2-input tensor ops on DVE only -- GPSIMD
    # tensor_tensor contends with DVE for the shared SBUF port (measured
    # ~2.4x mutual slowdown when both run).  GPSIMD gets only 1-input
    # tensor_scalar ops; ACT gets single-src affine/square/recip.
    NM = 37
    (D_, A_, R2, R1, B_, U_, C_, AR2, CR1, R1D, BR2, AD, BCp, N7, N6,
     DET, INV, INV2, E1, E0, Q_, R1A, T2, W2N, W2S, S6P, S3M, S2P, S7P,
     ES1, ES0, V1, V0, G3, G2, G5, G4) = range(NM)

    def act_recip(eng, out_ap, in_ap, scale):
        f32 = mybir.dt.float32
        eng.add_instruction(mybir.InstActivation(
            name=nc.get_next_instruction_name(),
            func=AF.Reciprocal,
            ins=[eng.lower_ap(in_ap),
                 mybir.ImmediateValue(dtype=f32, value=0.0),
                 mybir.ImmediateValue(dtype=f32, value=scale),
                 mybir.ImmediateValue(dtype=f32, value=0.0)],
            outs=[eng.lower_ap(out_ap)],
        ))

    with TileContext(nc, pool_alloc_mode="queue") as tc:
        with tc.tile_pool(name="io", bufs=3) as io, \
             tc.tile_pool(name="mid", bufs=2) as mid:
            for i in range(NT):
                X = io.tile([P, 8, T], bf16, tag="X", name="X")
                if i == 0:
                    # small first chunk {s1,s3,s2} unblocks the critical
                    # chain (q -> r1a -> r1) a couple of us earlier
                    nc.sync.dma_start(out=X[:, 0:3, :], in_=x[i, :, 0:3, :])
                    nc.sync.dma_start(out=X[:, 3:8, :], in_=x[i, :, 3:8, :])
                else:
                    nc.sync.dma_start(out=X, in_=x[i])
                Y = io.tile([P, 8, T], bf16, tag="Y", name="Y")
                M = mid.tile([P, NM, T], bf16, tag="M", name="M")

                def xs(c, w=1, step=1):
                    p = XP[c]
                    return X[:, p:p + 1 + (w - 1) * step:step, :]

                def m(p, w=1, step=1):
                    return M[:, p:p + 1 + (w - 1) * step:step, :]

                def ys(p, w=1):
                    return Y[:, p:p + w, :]

                V = nc.vector
                S = nc.scalar

                # ---- ACT: single-src affine / square / recip ----
                # shift planes keep the main chain in 2x-mode TT
                S.activation(m(S2P), xs(2), AF.Copy, bias=512.0, scale=1.0)
                S.activation(m(S3M), xs(3), AF.Copy, bias=-512.0, scale=1.0)
                S.activation(m(S7P), xs(7), AF.Copy, bias=512.0, scale=1.0)
                S.activation(m(S6P), xs(6), AF.Copy, bias=512.0, scale=1.0)
                # w2s = ((s5+512)/512)^2  (bias=1.0 is a registered const
                # AP); w2n = 512 - 512^2*w2s = 512 - (s5+512)^2
                S.activation(m(W2S), xs(5), AF.Square,
                             bias=1.0, scale=1.0 / 512)
                S.activation(m(W2N), m(W2S), AF.Copy,
                             bias=512.0, scale=-512.0 * 512.0)
                S.activation(m(G3), xs(1), AF.Copy, bias=0.0, scale=-1.0 / 512)
                S.activation(m(G2), xs(2), AF.Copy, bias=-1.0, scale=-1.0 / 512)
                S.activation(m(G5), xs(0), AF.Copy, bias=0.0, scale=1.0 / 512)
                S.activation(m(G4), xs(5), AF.Copy, bias=1.0, scale=1.0 / 512)

                # ---- DVE: everything else (all-bf16 unit-stride 2x TT) ----
                V.tensor_tensor(m(Q_), xs(3), xs(1), OP.subtract)
                V.tensor_tensor(m(R1A), m(S2P), m(Q_), OP.mult)
                V.tensor_tensor(m(E1), xs(1), xs(0), OP.subtract)
                V.tensor_tensor(m(E0), xs(2), xs(5), OP.subtract)
                V.tensor_tensor(m(T2), xs(0), xs(4), OP.mult)
                # [b, c] = [s2, s7] - [s6, s5]
                V.tensor_tensor(m(B_, 2, 2), xs(2, 2, 5), xs(6, 2, 1),
                                OP.subtract)
                # [d, a] = [s4, s7] - [s6p, s3m]
                V.tensor_tensor(m(D_, 2, 1), xs(4, 2, 4), m(S6P, 2, 1),
                                OP.subtract)
                V.tensor_tensor(m(R1), m(R1A), m(S7P), OP.subtract)
                V.tensor_tensor(m(U_), m(T2), xs(6), OP.add)
                # r2 = u + (512 - (s5+512)^2) = s0*s4 + s6 + 512 - (s5+512)^2
                V.tensor_tensor(m(R2), m(U_), m(W2N), OP.add)
                # [ar2, cr1] = [a, c] * [r2, r1]
                V.tensor_tensor(m(AR2, 2, 1), m(A_, 2, 5), m(R2, 2, 1),
                                OP.mult)
                # [r1d, br2] = [r1, b] * [d, r2]
                V.tensor_tensor(m(R1D, 2, 1), m(R1, 2, 1), m(D_, 2, 2),
                                OP.mult)
                # [ad, bc] = [a, b] * [d, c]
                V.tensor_tensor(m(AD, 2, 1), m(A_, 2, 3), m(D_, 2, 6),
                                OP.mult)
                # [n7, n6, det] = [ar2, r1d, ad] - [cr1, br2, bc]
                V.tensor_tensor(m(N7, 3, 1), m(AR2, 3, 2), m(CR1, 3, 2),
                                OP.subtract)
                act_recip(S, m(INV), m(DET), 512.0)
                act_recip(S, m(INV2), m(DET), 512.0)
                # [x7, x6] = [n7, n6] * [inv, inv2] -> Y planes 0,1
                V.tensor_tensor(ys(0, 2), m(N7, 2, 1), m(INV, 2, 1),
                                OP.mult)
                # [es1, es0] = [E1, E0]/512  (ACT, frees the STT)
                S.activation(M[:, ES1:ES1 + 2, :], m(E1, 2, 1), AF.Copy,
                             bias=0.0, scale=1.0 / 512)
                # [V1, V0] = [es1, es0] - [s3, s4]
                V.tensor_tensor(m(V1, 2, 1), m(ES1, 2, 1), xs(3, 2, 2),
                                OP.subtract)
                # [y1, y0] = [V1, V0] + [x7, x6] -> Y planes 2,3
                V.tensor_tensor(ys(2, 2), m(V1, 2, 1), ys(0, 2), OP.add)
                # [y3, y2, y5, y4] = [g3, g2, g5, g4] - [x7, x6, x7, x6]
                V.tensor_tensor(
                    Y[:, 4:8, :].rearrange("p (a b) t -> p a b t", b=2),
                    M[:, G3:G3 + 4, :].rearrange("p (a b) t -> p a b t", b=2),
                    Y[:, 0:2, :].unsqueeze(1).broadcast_to((P, 2, 2, T)),
                    OP.subtract)

                # stream finished planes out early on the last tile
                if i == NT - 1:
                    nc.sync.dma_start(out=y[i, :, 0:4, :], in_=Y[:, 0:4, :])
                    nc.sync.dma_start(out=y[i, :, 4:8, :], in_=Y[:, 4:8, :])
                else:
                    nc.sync.dma_start(out=y[i], in_=Y)
    if legalize:
        _legalize_waits(nc)
    return nc


def _legalize_waits(nc, max_waits=1):
    """HW instructions encode at most one semaphore wait; hoist extras
    onto NoOp carriers in the same engine queue."""
    import concourse.mybir as mybir

    skip = ("InstNoOp",)
    for f in nc.m.functions:
        for blk in f.blocks:
            il = blk.instructions
            out = []
            changed = False
            for inst in il:
                si = inst.sync_info
                if (si is not None and len(si.on_wait) > max_waits
                        and type(inst).__name__ not in skip):
                    waits = list(si.on_wait)
                    for w in waits[:-max_waits]:
                        out.append(mybir.InstNoOp(
                            name=nc.get_next_instruction_name(),
                            engine=inst.engine,
                            bass_nofuse=True,
                            sync_info=mybir.SyncInfo(
                                on_wait=[w], on_update=[]),
                        ))
                    inst.sync_info = mybir.SyncInfo(
                        on_wait=waits[-max_waits:],
                        on_update=list(si.on_update))
                    changed = True
                out.append(inst)
            if changed:
                blk.instructions = out


def _get_nc():
    if "nc" not in _CACHE:
        _CACHE["nc"] = _build_bass()
    return _CACHE["nc"]


def _run(shards, trace=False, **kwargs):
    from concourse.bass_utils import run_bass_kernel_spmd
    nc = _get_nc()
    in_maps = [{"x": s} for s in shards]
    return run_bass_kernel_spmd(
        nc, in_maps, core_ids=list(range(NCORES)), trace=trace, **kwargs)


def _prep_shards(pre_4pt_shift: np.ndarray):
    """(B, 8, 1) fp32 -> per-core planar bf16 shards (NT, P, 8, T)."""
    import ml_dtypes
    xf = np.asarray(pre_4pt_shift, dtype=np.float32).reshape(B_FULL, 8)
    xb = xf[:, XORDER].astype(ml_dtypes.bfloat16)
    xb = xb.reshape(NCORES, NT, P, T, 8).transpose(0, 1, 2, 4, 3)
    return [np.ascontiguousarray(xb[i]) for i in range(NCORES)]


def _assemble(results) -> np.ndarray:
    """per-core planar bf16 y planes -> (B, 3, 3) fp32 with ones col."""
    out = np.empty((B_FULL, 9), dtype=np.float32)
    out[:, 8] = 1.0
    for i in range(NCORES):
        yi = np.asarray(results[i]["y"]).astype(np.float32)  # (NT,P,8,T)
        out[i * BC:(i + 1) * BC, :8] = (
            yi.transpose(0, 1, 3, 2).reshape(BC, 8)[:, YINV])
    return out.reshape(B_FULL, 3, 3)


def kernel(pre_4pt_shift: np.ndarray) -> np.ndarray:
    shards = _prep_shards(pre_4pt_shift)
    r = _run(shards)
    return _assemble(r.results)


# revision 21
# speedup vs baseline: 1.1752x; 1.0681x over previous
"""Trainium2 Bass kernel for nn_DLTSolver (planar bf16 rewrite).

The reference solves, per batch element b (B = 1048576), an 8x8 linear
system that collapses analytically to a 2x2 Cramer solve plus affine
back-substitution (pure elementwise math in the 8 shift components
s0..s7):

    q  = s3 - s1            b = s2 - s6         c  = s7 - s5
    a  = (s7+512) - s3      d = (s4-512) - s6
    r1 = (s2+512)*q - (s7+512)
    r2 = s0*s4 + (s6+512) - (s5+512)^2
    det = a*d - b*c ;  inv = 1/(512*det)
    x6 = (r1*d - b*r2)*inv ;  x7 = (a*r2 - c*r1)*inv
    y0 = (s2-s5)/512 - s4 + x6   y1 = (s1-s0)/512 - s3 + x7
    y2 = -1 - s2/512 - x6        y3 = -s1/512 - x7
    y4 =  1 + s5/512 - x6        y5 =  s0/512 - x7
    out = [y0 y1 y2 y3 y4 y5 x6 x7 1] reshaped (3,3)

Layout strategy (the big change vs the interleaved kernel): the host
re-packs the input into PLANAR bf16 component planes, (tile, 128, 8
planes, T) per core, and the device writes planar bf16 output planes
[y0..y5, x6, x7].  The host re-interleaves + upcasts + appends the
constant ones column.  Consequences on device:
  - every engine op is a dense unit-stride plane op (no 32B-strided
    access tax, no (t,9) interleave tax),
  - all-bf16 tensor_tensor ops run in the DVE 2x_1P perf mode,
  - HBM traffic halves: 2 MiB in + 2 MiB out per core,
  - the ones column never touches the device.
Numerically verified in simulation: all-bf16 closed form gives
l2 rel err ~3.2e-3 vs the fp32 reference (gate is 2e-2).

Per-core work: 131072 elements = NT tiles x (128 partitions x T cols).
Engine split per tile: 16 DVE insts (packed 2-plane TT/STT), 6 GPSIMD
TT, 7 ACT (Square / Reciprocal spline / affine Copy), 2 HWDGE DMAs.

Input plane order (host-permuted so the packed 2-wide reads all have
non-negative plane steps): pos -> comp = [s0 s1 s2 s4 s3 s6 s5 s7].
"""

import numpy as np

P = 128
T = 512            # cols per partition per tile
NT = 2             # tiles per core
BC = P * T * NT    # elements per core = 131072
NCORES = 8
B_FULL = BC * NCORES

# plane position -> input component index (chosen so packed reads step >= 0
# and the critical-chain operands s1,s3,s2 ride in the first DMA chunk)
XORDER = [1, 3, 2, 4, 0, 6, 5, 7]
# device output plane order (x7,x6 finish first, then y1,y0, then y3,y2,y5,y4)
YORDER = [7, 6, 1, 0, 3, 2, 5, 4]  # plane position -> h-index
# YINV[h] = plane position holding h
YINV = [YORDER.index(h) for h in range(8)]
# X plane positions by component
XP = {c: p for p, c in enumerate(XORDER)}

_CACHE: dict = {}


def _build_bass(legalize=True):
    import concourse.bass as bass
    import concourse.mybir as mybir
    from concourse.tile import TileContext

    bf16 = mybir.dt.bfloat16
    OP = mybir.AluOpType
    AF = mybir.ActivationFunctionType

    nc = bass.Bass("TRN2", use_seq_codegen=True)
    x = nc.dram_tensor("x", [NT, P, 8, T], bf16, kind="ExternalInput")
    y = nc.dram_tensor("y", [NT, P, 8, T], bf16, kind="ExternalOutput")

    # mid-plane layout (chosen so every packed read/write AP has a
    # non-negative plane step); 2-input tensor ops on DVE only -- GPSIMD
    # tensor_tensor contends with DVE for the shared SBUF port (measured
    # ~2.4x mutual slowdown when both run).  GPSIMD gets only 1-input
    # tensor_scalar ops; ACT gets single-src affine/square/recip.
    NM = 37
    (D_, A_, R2, R1, B_, U_, C_, AR2, CR1, R1D, BR2, AD, BCp, N7, N6,
     DET, INV, INV2, E1, E0, Q_, R1A, T2, W2N, W2S, S6P, S3M, S2P, S7P,
     ES1, ES0, V1, V0, G3, G2, G5, G4) = range(NM)

    def act_recip(eng, out_ap, in_ap, scale):
        f32 = mybir.dt.float32
        eng.add_instruction(mybir.InstActivation(
            name=nc.get_next_instruction_name(),
            func=AF.Reciprocal,
            ins=[eng.lower_ap(in_ap),
                 mybir.ImmediateValue(dtype=f32, value=0.0),
                 mybir.ImmediateValue(dtype=f32, value=scale),
                 mybir.ImmediateValue(dtype=f32, value=0.0)],
            outs=[eng.lower_ap(out_ap)],
        ))

    with TileContext(nc, pool_alloc_mode="queue") as tc:
        with tc.tile_pool(name="io", bufs=3) as io, \
             tc.tile_pool(name="mid", bufs=2) as mid:
            for i in range(NT):
                X = io.tile([P, 8, T], bf16, tag="X", name="X")
                if i == 0:
                    # first chunk {s1,s3,s2,s4,s0} unblocks q, r1a, E1, t2
                    # while {s6,s5,s7} are still in flight
                    nc.sync.dma_start(out=X[:, 0:5, :], in_=x[i, :, 0:5, :])
                    nc.sync.dma_start(out=X[:, 5:8, :], in_=x[i, :, 5:8, :])
                else:
                    nc.sync.dma_start(out=X, in_=x[i])
                Y = io.tile([P, 8, T], bf16, tag="Y", name="Y")
                M = mid.tile([P, NM, T], bf16, tag="M", name="M")

                def xs(c, w=1, step=1):
                    p = XP[c]
                    return X[:, p:p + 1 + (w - 1) * step:step, :]

                def m(p, w=1, step=1):
                    return M[:, p:p + 1 + (w - 1) * step:step, :]

                def ys(p, w=1):
                    return Y[:, p:p + w, :]

                V = nc.vector
                S = nc.scalar

                # ---- ACT: single-src affine / square / recip ----
                # shift planes keep the main chain in 2x-mode TT
                S.activation(m(S2P), xs(2), AF.Copy, bias=512.0, scale=1.0)
                S.activation(m(S3M), xs(3), AF.Copy, bias=-512.0, scale=1.0)
                S.activation(m(S7P), xs(7), AF.Copy, bias=512.0, scale=1.0)
                S.activation(m(S6P), xs(6), AF.Copy, bias=512.0, scale=1.0)
                # w2s = ((s5+512)/512)^2  (bias=1.0 is a registered const
                # AP); w2n = 512 - 512^2*w2s = 512 - (s5+512)^2
                S.activation(m(W2S), xs(5), AF.Square,
                             bias=1.0, scale=1.0 / 512)
                S.activation(m(W2N), m(W2S), AF.Copy,
                             bias=512.0, scale=-512.0 * 512.0)
                S.activation(m(G3), xs(1), AF.Copy, bias=0.0, scale=-1.0 / 512)
                S.activation(m(G2), xs(2), AF.Copy, bias=-1.0, scale=-1.0 / 512)
                S.activation(m(G5), xs(0), AF.Copy, bias=0.0, scale=1.0 / 512)
                S.activation(m(G4), xs(5), AF.Copy, bias=1.0, scale=1.0 / 512)

                # ---- DVE: everything else (all-bf16 unit-stride 2x TT) ----
                # chunk1-only ops first (s1,s3,s2,s4,s0 arrive early)
                V.tensor_tensor(m(Q_), xs(3), xs(1), OP.subtract)
                V.tensor_tensor(m(R1A), m(S2P), m(Q_), OP.mult)
                V.tensor_tensor(m(E1), xs(1), xs(0), OP.subtract)
                V.tensor_tensor(m(T2), xs(0), xs(4), OP.mult)
                V.tensor_tensor(m(E0), xs(2), xs(5), OP.subtract)
                # [b, c] = [s2, s7] - [s6, s5]
                V.tensor_tensor(m(B_, 2, 2), xs(2, 2, 5), xs(6, 2, 1),
                                OP.subtract)
                # [d, a] = [s4, s7] - [s6p, s3m]
                V.tensor_tensor(m(D_, 2, 1), xs(4, 2, 4), m(S6P, 2, 1),
                                OP.subtract)
                V.tensor_tensor(m(R1), m(R1A), m(S7P), OP.subtract)
                V.tensor_tensor(m(U_), m(T2), xs(6), OP.add)
                # r2 = u + (512 - (s5+512)^2) = s0*s4 + s6 + 512 - (s5+512)^2
                V.tensor_tensor(m(R2), m(U_), m(W2N), OP.add)
                # [ar2, cr1] = [a, c] * [r2, r1]
                V.tensor_tensor(m(AR2, 2, 1), m(A_, 2, 5), m(R2, 2, 1),
                                OP.mult)
                # [r1d, br2] = [r1, b] * [d, r2]
                V.tensor_tensor(m(R1D, 2, 1), m(R1, 2, 1), m(D_, 2, 2),
                                OP.mult)
                # [ad, bc] = [a, b] * [d, c]
                V.tensor_tensor(m(AD, 2, 1), m(A_, 2, 3), m(D_, 2, 6),
                                OP.mult)
                # [n7, n6, det] = [ar2, r1d, ad] - [cr1, br2, bc]
                V.tensor_tensor(m(N7, 3, 1), m(AR2, 3, 2), m(CR1, 3, 2),
                                OP.subtract)
                act_recip(S, m(INV), m(DET), 512.0)
                # [x7, x6] = [n7, n6] * inv (broadcast) -> Y planes 0,1
                V.tensor_tensor(ys(0, 2), m(N7, 2, 1),
                                m(INV).broadcast_to((P, 2, T)), OP.mult)
                # [es1, es0] = [E1, E0]/512  (ACT, frees the STT)
                S.activation(M[:, ES1:ES1 + 2, :], m(E1, 2, 1), AF.Copy,
                             bias=0.0, scale=1.0 / 512)
                # [V1, V0] = [es1, es0] - [s3, s4]
                V.tensor_tensor(m(V1, 2, 1), m(ES1, 2, 1), xs(3, 2, 2),
                                OP.subtract)
                # [y1, y0] = [V1, V0] + [x7, x6] -> Y planes 2,3
                V.tensor_tensor(ys(2, 2), m(V1, 2, 1), ys(0, 2), OP.add)
                # [y3, y2, y5, y4] = [g3, g2, g5, g4] - [x7, x6, x7, x6]
                V.tensor_tensor(
                    Y[:, 4:8, :].rearrange("p (a b) t -> p a b t", b=2),
                    M[:, G3:G3 + 4, :].rearrange("p (a b) t -> p a b t", b=2),
                    Y[:, 0:2, :].unsqueeze(1).broadcast_to((P, 2, 2, T)),
                    OP.subtract)

                # stream finished planes out early on the last tile
                if i == NT - 1:
                    nc.sync.dma_start(out=y[i, :, 0:4, :], in_=Y[:, 0:4, :])
                    nc.sync.dma_start(out=y[i, :, 4:8, :], in_=Y[:, 4:8, :])
                else:
                    nc.sync.dma_start(out=y[i], in_=Y)
    if legalize:
        _legalize_waits(nc)
    return nc


def _legalize_waits(nc, max_waits=1):
    """HW instructions encode at most one semaphore wait; hoist extras
    onto NoOp carriers in the same engine queue."""
    import concourse.mybir as mybir

    skip = ("InstNoOp",)
    for f in nc.m.functions:
        for blk in f.blocks:
            il = blk.instructions
            out = []
            changed = False
            for inst in il:
                si = inst.sync_info
                if (si is not None and len(si.on_wait) > max_waits
                        and type(inst).__name__ not in skip):
                    waits = list(si.on_wait)
                    for w in waits[:-max_waits]:
                        out.append(mybir.InstNoOp(
                            name=nc.get_next_instruction_name(),
                            engine=inst.engine,
                            bass_nofuse=True,
                            sync_info=mybir.SyncInfo(
                                on_wait=[w], on_update=[]),
                        ))
                    inst.sync_info = mybir.SyncInfo(
                        on_wait=waits[-max_waits:],
                        on_update=list(si.on_update))
                    changed = True
                out.append(inst)
            if changed:
                blk.instructions = out


def _get_nc():
    if "nc" not in _CACHE:
        _CACHE["nc"] = _build_bass()
    return _CACHE["nc"]


def _run(shards, trace=False, **kwargs):
    from concourse.bass_utils import run_bass_kernel_spmd
    nc = _get_nc()
    in_maps = [{"x": s} for s in shards]
    return run_bass_kernel_spmd(
        nc, in_maps, core_ids=list(range(NCORES)), trace=trace, **kwargs)


def _prep_shards(pre_4pt_shift: np.ndarray):
    """(B, 8, 1) fp32 -> per-core planar bf16 shards (NT, P, 8, T)."""
    import ml_dtypes
    xf = np.asarray(pre_4pt_shift, dtype=np.float32).reshape(B_FULL, 8)
    xb = xf[:, XORDER].astype(ml_dtypes.bfloat16)
    xb = xb.reshape(NCORES, NT, P, T, 8).transpose(0, 1, 2, 4, 3)
    return [np.ascontiguousarray(xb[i]) for i in range(NCORES)]


def _assemble(results) -> np.ndarray:
    """per-core planar bf16 y planes -> (B, 3, 3) fp32 with ones col."""
    out = np.empty((B_FULL, 9), dtype=np.float32)
    out[:, 8] = 1.0
    for i in range(NCORES):
        yi = np.asarray(results[i]["y"]).astype(np.float32)  # (NT,P,8,T)
        out[i * BC:(i + 1) * BC, :8] = (
            yi.transpose(0, 1, 3, 2).reshape(BC, 8)[:, YINV])
    return out.reshape(B_FULL, 3, 3)


def kernel(pre_4pt_shift: np.ndarray) -> np.ndarray:
    shards = _prep_shards(pre_4pt_shift)
    r = _run(shards)
    return _assemble(r.results)
